# revision 4
# baseline (speedup 1.0000x reference)
"""Trainium2 Bass kernel for ContextAwareArtRecSys (gnn_message_passing).

Math fold: the reference is
    score[e] = concat(z_u[src] @ Wu.T + bu, z_i[dst] @ Wi.T + bi) @ wo.T + bo
Everything after the gather is linear, so with
    vu = wo[:, :128] @ Wu          (256-vector)
    vi = wo[:, 128:] @ Wi          (256-vector)
    c  = wo[:, :128]@bu + wo[:, 128:]@bi + bo   (scalar)
we have score[e] = (z_u @ vu)[src] + (z_i @ vi)[dst] + c.

Device plan per core k of 8 (SPMD):
  1. fold vu/vi/c on PE from the raw weights (replicated inputs).
  2. matvec s_u = z_u_shard @ vu + c, s_i = z_i_shard @ vi on DVE while
     the z shards stream in (z is sharded 8-ways by node).
  3. AllGather the scalar tables to DRAM (50000 + 100000 f32).
  4. per-element indirect-DMA gather of s_u[src_e] and s_i[dst_e] for the
     core's 1/8 slice of the edge list, add, store.
All heavy work is on-device; the host only slices/casts/pads/lays out
shards and concatenates the output.
"""

import numpy as np

N_CORES = 8
N_USERS, N_ITEMS, E, H = 50000, 100000, 500000, 256
HALF = H // 2

U_SH = N_USERS // N_CORES          # 6250 users per core
I_SH = N_ITEMS // N_CORES          # 12500 items per core
E_SH = E // N_CORES                # 62500 edges per core
U_PAD = 6272                       # 49 * 128
I_PAD = 12544                      # 98 * 128
U_TILES = U_PAD // 128             # 49
I_TILES = I_PAD // 128             # 98

N_GI = 16                          # gather instructions per table
CP = 3968                          # elements gathered per instruction (31*128)
COLS = CP // 128                   # 31 idx columns per instruction
E_PAD = N_GI * CP                  # 63488 padded edge slots per core

_CACHE = {}


def _build():
    if "nc" in _CACHE:
        return _CACHE["nc"]
    import concourse.bass as bass
    import concourse.tile as tile
    import concourse.mybir as mybir
    from concourse import bacc
    from concourse.bass import IndirectOffsetOnAxis

    f32 = mybir.dt.float32
    i32 = mybir.dt.int32

    nc = bacc.Bacc("TRN2", target_bir_lowering=False, debug=False,
                   num_devices=N_CORES)

    zu = nc.dram_tensor("zu", [U_PAD, H], f32, kind="ExternalInput")
    zi = nc.dram_tensor("zi", [I_PAD, H], f32, kind="ExternalInput")
    w_user = nc.dram_tensor("w_user", [HALF, H], f32, kind="ExternalInput")
    w_item = nc.dram_tensor("w_item", [HALF, H], f32, kind="ExternalInput")
    wo_u = nc.dram_tensor("wo_u", [HALF, 1], f32, kind="ExternalInput")
    wo_i = nc.dram_tensor("wo_i", [HALF, 1], f32, kind="ExternalInput")
    b_user = nc.dram_tensor("b_user", [HALF, 1], f32, kind="ExternalInput")
    b_item = nc.dram_tensor("b_item", [HALF, 1], f32, kind="ExternalInput")
    b_out = nc.dram_tensor("b_out", [1, 1], f32, kind="ExternalInput")
    idxu = nc.dram_tensor("idxu", [128, N_GI * COLS], i32, kind="ExternalInput")
    idxi = nc.dram_tensor("idxi", [128, N_GI * COLS], i32, kind="ExternalInput")
    out = nc.dram_tensor("out", [N_GI, CP], f32, kind="ExternalOutput")

    s_uc = nc.dram_tensor("s_uc", [U_SH, 1], f32)
    s_ic = nc.dram_tensor("s_ic", [I_SH, 1], f32)
    s_uf = nc.dram_tensor("s_uf", [N_USERS, 1], f32, addr_space="Shared")
    s_if = nc.dram_tensor("s_if", [N_ITEMS, 1], f32, addr_space="Shared")

    groups = [list(range(N_CORES))]

    with tile.TileContext(nc) as tc:
        with (
            tc.tile_pool(name="consts", bufs=1) as consts,
            tc.tile_pool(name="zpool", bufs=4) as zpool,
            tc.tile_pool(name="spool", bufs=1) as spool,
            tc.tile_pool(name="gpool", bufs=1) as gpool,
            tc.tile_pool(name="psum", bufs=2, space="PSUM") as psum,
        ):
            # ---- fold vu / vi / c on PE ----
            wu_t = consts.tile([HALF, H], f32)
            nc.sync.dma_start(wu_t[:], w_user.ap())
            wi_t = consts.tile([HALF, H], f32)
            nc.sync.dma_start(wi_t[:], w_item.ap())
            wou_t = consts.tile([HALF, 1], f32)
            nc.sync.dma_start(wou_t[:], wo_u.ap())
            woi_t = consts.tile([HALF, 1], f32)
            nc.sync.dma_start(woi_t[:], wo_i.ap())
            bu_t = consts.tile([HALF, 1], f32)
            nc.sync.dma_start(bu_t[:], b_user.ap())
            bi_t = consts.tile([HALF, 1], f32)
            nc.sync.dma_start(bi_t[:], b_item.ap())
            bo_t = consts.tile([1, 1], f32)
            nc.sync.dma_start(bo_t[:], b_out.ap())

            # replicate wo halves across the free dim: rep[k, m] = wo[k]
            ones_kk = consts.tile([HALF, HALF], f32)
            nc.vector.memset(ones_kk[:], 1.0)
            wou_rep = consts.tile([HALF, HALF], f32)
            nc.vector.tensor_scalar_mul(wou_rep[:], ones_kk[:], wou_t[:])
            woi_rep = consts.tile([HALF, HALF], f32)
            nc.vector.tensor_scalar_mul(woi_rep[:], ones_kk[:], woi_t[:])

            # vu/vi broadcast across all 128 partitions: [128, H] PSUM
            vu_ps = psum.tile([HALF, H], f32, tag="vps")
            nc.tensor.matmul(vu_ps[:], wou_rep[:], wu_t[:], start=True, stop=True)
            vu_t = consts.tile([HALF, H], f32)
            nc.vector.tensor_copy(vu_t[:], vu_ps[:])
            vi_ps = psum.tile([HALF, H], f32, tag="vps")
            nc.tensor.matmul(vi_ps[:], woi_rep[:], wi_t[:], start=True, stop=True)
            vi_t = consts.tile([HALF, H], f32)
            nc.vector.tensor_copy(vi_t[:], vi_ps[:])

            # c = wo_u . b_user + wo_i . b_item + b_out, broadcast to [128,1]
            ones_k1 = consts.tile([HALF, 128], f32)
            nc.vector.memset(ones_k1[:], 1.0)
            cu_ps = psum.tile([128, 1], f32, tag="cps")
            bub = consts.tile([HALF, 128], f32)
            nc.vector.tensor_scalar_mul(bub[:], ones_k1[:], bu_t[:])
            bib = consts.tile([HALF, 128], f32)
            nc.vector.tensor_scalar_mul(bib[:], ones_k1[:], bi_t[:])
            # cu_ps[m, 0] = sum_k bub[k, m] * wo_u[k]  (same for all m)
            nc.tensor.matmul(cu_ps[:], bub[:], wou_t[:], start=True, stop=False)
            nc.tensor.matmul(cu_ps[:], bib[:], woi_t[:], start=False, stop=False)
            # += 1 * b_out via a K=1 matmul (ones row as lhsT)
            nc.tensor.matmul(
                cu_ps[:], ones_k1[0:1, :], bo_t[:], start=False, stop=True
            )
            c_t = consts.tile([128, 1], f32)
            nc.vector.tensor_copy(c_t[:], cu_ps[:])

            # ---- phase 1: s tables (DVE matvec over streamed z tiles) ----
            su_sb = spool.tile([128, U_TILES], f32)
            for t in range(U_TILES):
                zt = zpool.tile([128, H], f32, tag="z")
                nc.sync.dma_start(zt[:], zu.ap()[t * 128:(t + 1) * 128, :])
                pr = zpool.tile([128, H], f32, tag="prod")
                nc.vector.tensor_mul(pr[:], zt[:], vu_t[:])
                nc.vector.reduce_sum(
                    su_sb[:, t:t + 1], pr[:], axis=mybir.AxisListType.X
                )
            # add folded constant c into the user table
            nc.vector.tensor_scalar_add(su_sb[:], su_sb[:], c_t[:])

            si_sb = spool.tile([128, I_TILES], f32)
            for t in range(I_TILES):
                zt = zpool.tile([128, H], f32, tag="z")
                nc.sync.dma_start(zt[:], zi.ap()[t * 128:(t + 1) * 128, :])
                pr = zpool.tile([128, H], f32, tag="prod")
                nc.vector.tensor_mul(pr[:], zt[:], vi_t[:])
                nc.vector.reduce_sum(
                    si_sb[:, t:t + 1], pr[:], axis=mybir.AxisListType.X
                )

            # ---- store chunks to DRAM (node-major order), allgather ----
            # s_uc[n] for n = 128*t + p  <=>  sbuf [p, t]
            n_full_u = U_SH // 128              # 48 full tiles
            rem_u = U_SH - n_full_u * 128       # 106
            nc.sync.dma_start(
                s_uc.ap()[: n_full_u * 128, :].rearrange(
                    "(t p) one -> p (t one)", p=128
                ),
                su_sb[:, :n_full_u],
            )
            nc.sync.dma_start(
                s_uc.ap()[n_full_u * 128:, :],
                su_sb[:rem_u, n_full_u:n_full_u + 1],
            )
            n_full_i = I_SH // 128              # 97
            rem_i = I_SH - n_full_i * 128       # 84
            nc.sync.dma_start(
                s_ic.ap()[: n_full_i * 128, :].rearrange(
                    "(t p) one -> p (t one)", p=128
                ),
                si_sb[:, :n_full_i],
            )
            nc.sync.dma_start(
                s_ic.ap()[n_full_i * 128:, :],
                si_sb[:rem_i, n_full_i:n_full_i + 1],
            )

            nc.gpsimd.collective_compute(
                "AllGather", mybir.AluOpType.bypass,
                replica_groups=groups, ins=[s_uc.ap()], outs=[s_uf.ap()],
            )
            nc.gpsimd.collective_compute(
                "AllGather", mybir.AluOpType.bypass,
                replica_groups=groups, ins=[s_ic.ap()], outs=[s_if.ap()],
            )

            # ---- phase 2: per-element indirect gathers ----
            idxu_t = gpool.tile([128, N_GI * COLS], i32)
            nc.sync.dma_start(idxu_t[:], idxu.ap())
            idxi_t = gpool.tile([128, N_GI * COLS], i32)
            nc.sync.dma_start(idxi_t[:], idxi.ap())

            # all user gathers FIRST: gpsimd runs in order, and the user
            # table is ready well before the item table. Each batch sits in
            # a critical section with a manual completion semaphore so the
            # Q7 generates descriptors back-to-back instead of waiting for
            # each gather's DMA to drain.
            gu_t = gpool.tile([128, CP], f32)
            gi_t = gpool.tile([128, CP], f32)
            sem_g = nc.alloc_semaphore("gsem")
            with tc.tile_critical():
                for p in range(N_GI):
                    nc.gpsimd.indirect_dma_start(
                        out=gu_t[p:p + 1, :].rearrange(
                            "one (c x) -> one c x", x=1
                        ),
                        out_offset=None,
                        in_=s_uf.ap(),
                        in_offset=IndirectOffsetOnAxis(
                            ap=idxu_t[:, p * COLS:(p + 1) * COLS], axis=0
                        ),
                    ).then_inc(sem_g, 16)
            with tc.tile_critical():
                for p in range(N_GI):
                    nc.gpsimd.indirect_dma_start(
                        out=gi_t[p:p + 1, :].rearrange(
                            "one (c x) -> one c x", x=1
                        ),
                        out_offset=None,
                        in_=s_if.ap(),
                        in_offset=IndirectOffsetOnAxis(
                            ap=idxi_t[:, p * COLS:(p + 1) * COLS], axis=0
                        ),
                    ).then_inc(sem_g, 16)

            sc_t = gpool.tile([N_GI, CP], f32)
            with tc.tile_critical():
                nc.vector.wait_ge(sem_g, 2 * N_GI * 16)
                nc.vector.tensor_add(
                    sc_t[:], gu_t[:N_GI, :], gi_t[:N_GI, :]
                )
            nc.sync.dma_start(out.ap(), sc_t[:])

    nc.compile()
    _CACHE["nc"] = nc
    return nc


def _wrap_idx(vals):
    """Lay a per-core int32 index stream out for the gather instructions.

    Instruction p consumes its [128, COLS] idx slice in spray order
    (partition-fastest), writing element j of its row; so slice columns
    hold vals[p*CP : (p+1)*CP] reshaped (COLS, 128) transposed.
    """
    full = np.zeros(E_PAD, dtype=np.int32)
    full[: len(vals)] = vals
    outm = np.empty((128, N_GI * COLS), dtype=np.int32)
    for p in range(N_GI):
        seg = full[p * CP:(p + 1) * CP]
        outm[:, p * COLS:(p + 1) * COLS] = seg.reshape(COLS, 128).T
    return outm


def _make_in_maps(inputs):
    z_user = np.ascontiguousarray(np.asarray(inputs["z_user"], dtype=np.float32))
    z_item = np.ascontiguousarray(np.asarray(inputs["z_item"], dtype=np.float32))
    src = np.asarray(inputs["edge_src"]).astype(np.int32)
    dst = np.asarray(inputs["edge_dst"]).astype(np.int32)
    w_user = np.asarray(inputs["w_user"], dtype=np.float32)
    w_item = np.asarray(inputs["w_item"], dtype=np.float32)
    b_user = np.asarray(inputs["b_user"], dtype=np.float32).reshape(HALF, 1)
    b_item = np.asarray(inputs["b_item"], dtype=np.float32).reshape(HALF, 1)
    w_out = np.asarray(inputs["w_out"], dtype=np.float32)
    b_out = np.asarray(inputs["b_out"], dtype=np.float32).reshape(1, 1)
    wo_u = w_out[0, :HALF].reshape(HALF, 1).copy()
    wo_i = w_out[0, HALF:].reshape(HALF, 1).copy()

    in_maps = []
    for k in range(N_CORES):
        zu_k = np.zeros((U_PAD, H), dtype=np.float32)
        zu_k[:U_SH] = z_user[k * U_SH:(k + 1) * U_SH]
        zi_k = np.zeros((I_PAD, H), dtype=np.float32)
        zi_k[:I_SH] = z_item[k * I_SH:(k + 1) * I_SH]
        in_maps.append({
            "zu": zu_k,
            "zi": zi_k,
            "w_user": w_user,
            "w_item": w_item,
            "wo_u": wo_u,
            "wo_i": wo_i,
            "b_user": b_user,
            "b_item": b_item,
            "b_out": b_out,
            "idxu": _wrap_idx(src[k * E_SH:(k + 1) * E_SH]),
            "idxi": _wrap_idx(dst[k * E_SH:(k + 1) * E_SH]),
        })
    return in_maps


def _run(inputs, trace=False):
    from concourse.bass_utils import run_bass_kernel_spmd

    nc = _build()
    in_maps = _make_in_maps(inputs)
    res = run_bass_kernel_spmd(
        nc, in_maps, core_ids=list(range(N_CORES)), trace=trace
    )
    parts = [res.results[k]["out"].reshape(-1)[:E_SH] for k in range(N_CORES)]
    full = np.concatenate(parts).reshape(E, 1).astype(np.float32)
    return full, res


def kernel(**inputs):
    full, _ = _run(inputs, trace=False)
    return full


# revision 5
# speedup vs baseline: 1.1240x; 1.1240x over previous
"""Trainium2 Bass kernel for ContextAwareArtRecSys (gnn_message_passing).

Math fold: the reference is
    score[e] = concat(z_u[src] @ Wu.T + bu, z_i[dst] @ Wi.T + bi) @ wo.T + bo
Everything after the gather is linear, so with
    vu = wo[:, :128] @ Wu          (256-vector)
    vi = wo[:, 128:] @ Wi          (256-vector)
    c  = wo[:, :128]@bu + wo[:, 128:]@bi + bo   (scalar)
we have score[e] = (z_u @ vu)[src] + (z_i @ vi)[dst] + c.

Device plan per core k of 8 (SPMD):
  1. fold vu/vi/c on PE from the raw weights (replicated inputs).
  2. matvec s_u = z_u_shard @ vu + c, s_i = z_i_shard @ vi on DVE while
     the z shards stream in (z is sharded 8-ways by node).
  3. AllGather the scalar tables to DRAM (50000 + 100000 f32).
  4. per-element indirect-DMA gather of s_u[src_e] and s_i[dst_e] for the
     core's 1/8 slice of the edge list, add, store.
All heavy work is on-device; the host only slices/casts/pads/lays out
shards and concatenates the output.
"""

import numpy as np

N_CORES = 8
N_USERS, N_ITEMS, E, H = 50000, 100000, 500000, 256
HALF = H // 2

U_SH = N_USERS // N_CORES          # 6250 users per core
I_SH = N_ITEMS // N_CORES          # 12500 items per core
E_SH = E // N_CORES                # 62500 edges per core
U_PAD = 6272                       # 49 * 128
I_PAD = 12544                      # 98 * 128
U_TILES = U_PAD // 128             # 49
I_TILES = I_PAD // 128             # 98

N_GI = 16                          # gather instructions per table
CP = 3968                          # elements gathered per instruction (31*128)
COLS = CP // 128                   # 31 idx columns per instruction
E_PAD = N_GI * CP                  # 63488 padded edge slots per core

_CACHE = {}


def _build():
    if "nc" in _CACHE:
        return _CACHE["nc"]
    import concourse.bass as bass
    import concourse.tile as tile
    import concourse.mybir as mybir
    from concourse import bacc
    from concourse.bass import IndirectOffsetOnAxis

    f32 = mybir.dt.float32
    i32 = mybir.dt.int32

    nc = bacc.Bacc("TRN2", target_bir_lowering=False, debug=False,
                   num_devices=N_CORES)

    zu = nc.dram_tensor("zu", [U_PAD, H], f32, kind="ExternalInput")
    zi = nc.dram_tensor("zi", [I_PAD, H], f32, kind="ExternalInput")
    w_user = nc.dram_tensor("w_user", [HALF, H], f32, kind="ExternalInput")
    w_item = nc.dram_tensor("w_item", [HALF, H], f32, kind="ExternalInput")
    wo_u = nc.dram_tensor("wo_u", [HALF, 1], f32, kind="ExternalInput")
    wo_i = nc.dram_tensor("wo_i", [HALF, 1], f32, kind="ExternalInput")
    b_user = nc.dram_tensor("b_user", [HALF, 1], f32, kind="ExternalInput")
    b_item = nc.dram_tensor("b_item", [HALF, 1], f32, kind="ExternalInput")
    b_out = nc.dram_tensor("b_out", [1, 1], f32, kind="ExternalInput")
    idxu = nc.dram_tensor("idxu", [128, N_GI * COLS], i32, kind="ExternalInput")
    idxi = nc.dram_tensor("idxi", [128, N_GI * COLS], i32, kind="ExternalInput")
    out = nc.dram_tensor("out", [N_GI, CP], f32, kind="ExternalOutput")

    s_uc = nc.dram_tensor("s_uc", [U_SH, 1], f32)
    s_ic = nc.dram_tensor("s_ic", [I_SH, 1], f32)
    s_uf = nc.dram_tensor("s_uf", [N_USERS, 1], f32, addr_space="Shared")
    s_if = nc.dram_tensor("s_if", [N_ITEMS, 1], f32, addr_space="Shared")

    groups = [list(range(N_CORES))]

    with tile.TileContext(nc) as tc:
        with (
            tc.tile_pool(name="consts", bufs=1) as consts,
            tc.tile_pool(name="zpool", bufs=4) as zpool,
            tc.tile_pool(name="spool", bufs=1) as spool,
            tc.tile_pool(name="gpool", bufs=1) as gpool,
            tc.tile_pool(name="psum", bufs=2, space="PSUM") as psum,
        ):
            # ---- fold vu / vi / c on PE ----
            wu_t = consts.tile([HALF, H], f32)
            nc.sync.dma_start(wu_t[:], w_user.ap())
            wi_t = consts.tile([HALF, H], f32)
            nc.sync.dma_start(wi_t[:], w_item.ap())
            wou_t = consts.tile([HALF, 1], f32)
            nc.sync.dma_start(wou_t[:], wo_u.ap())
            woi_t = consts.tile([HALF, 1], f32)
            nc.sync.dma_start(woi_t[:], wo_i.ap())
            bu_t = consts.tile([HALF, 1], f32)
            nc.sync.dma_start(bu_t[:], b_user.ap())
            bi_t = consts.tile([HALF, 1], f32)
            nc.sync.dma_start(bi_t[:], b_item.ap())
            bo_t = consts.tile([1, 1], f32)
            nc.sync.dma_start(bo_t[:], b_out.ap())

            # replicate wo halves across the free dim: rep[k, m] = wo[k]
            ones_kk = consts.tile([HALF, HALF], f32)
            nc.vector.memset(ones_kk[:], 1.0)
            wou_rep = consts.tile([HALF, HALF], f32)
            nc.vector.tensor_scalar_mul(wou_rep[:], ones_kk[:], wou_t[:])
            woi_rep = consts.tile([HALF, HALF], f32)
            nc.vector.tensor_scalar_mul(woi_rep[:], ones_kk[:], woi_t[:])

            # vu/vi broadcast across all 128 partitions: [128, H] PSUM
            vu_ps = psum.tile([HALF, H], f32, tag="vps")
            nc.tensor.matmul(vu_ps[:], wou_rep[:], wu_t[:], start=True, stop=True)
            vu_t = consts.tile([HALF, H], f32)
            nc.vector.tensor_copy(vu_t[:], vu_ps[:])
            vi_ps = psum.tile([HALF, H], f32, tag="vps")
            nc.tensor.matmul(vi_ps[:], woi_rep[:], wi_t[:], start=True, stop=True)
            vi_t = consts.tile([HALF, H], f32)
            nc.vector.tensor_copy(vi_t[:], vi_ps[:])

            # c = wo_u . b_user + wo_i . b_item + b_out, broadcast to [128,1]
            ones_k1 = consts.tile([HALF, 128], f32)
            nc.vector.memset(ones_k1[:], 1.0)
            cu_ps = psum.tile([128, 1], f32, tag="cps")
            bub = consts.tile([HALF, 128], f32)
            nc.vector.tensor_scalar_mul(bub[:], ones_k1[:], bu_t[:])
            bib = consts.tile([HALF, 128], f32)
            nc.vector.tensor_scalar_mul(bib[:], ones_k1[:], bi_t[:])
            # cu_ps[m, 0] = sum_k bub[k, m] * wo_u[k]  (same for all m)
            nc.tensor.matmul(cu_ps[:], bub[:], wou_t[:], start=True, stop=False)
            nc.tensor.matmul(cu_ps[:], bib[:], woi_t[:], start=False, stop=False)
            # += 1 * b_out via a K=1 matmul (ones row as lhsT)
            nc.tensor.matmul(
                cu_ps[:], ones_k1[0:1, :], bo_t[:], start=False, stop=True
            )
            c_t = consts.tile([128, 1], f32)
            nc.vector.tensor_copy(c_t[:], cu_ps[:])

            # ---- phase 1: s tables (DVE matvec over streamed z tiles) ----
            su_sb = spool.tile([128, U_TILES], f32)
            for t in range(U_TILES):
                zt = zpool.tile([128, H], f32, tag="z")
                nc.sync.dma_start(zt[:], zu.ap()[t * 128:(t + 1) * 128, :])
                pr = zpool.tile([128, H], f32, tag="prod")
                nc.vector.tensor_mul(pr[:], zt[:], vu_t[:])
                nc.vector.reduce_sum(
                    su_sb[:, t:t + 1], pr[:], axis=mybir.AxisListType.X
                )
            # add folded constant c into the user table
            nc.vector.tensor_scalar_add(su_sb[:], su_sb[:], c_t[:])

            si_sb = spool.tile([128, I_TILES], f32)
            for t in range(I_TILES):
                zt = zpool.tile([128, H], f32, tag="z")
                nc.sync.dma_start(zt[:], zi.ap()[t * 128:(t + 1) * 128, :])
                pr = zpool.tile([128, H], f32, tag="prod")
                nc.vector.tensor_mul(pr[:], zt[:], vi_t[:])
                nc.vector.reduce_sum(
                    si_sb[:, t:t + 1], pr[:], axis=mybir.AxisListType.X
                )

            # ---- store chunks to DRAM (node-major order), allgather ----
            # s_uc[n] for n = 128*t + p  <=>  sbuf [p, t]
            n_full_u = U_SH // 128              # 48 full tiles
            rem_u = U_SH - n_full_u * 128       # 106
            nc.sync.dma_start(
                s_uc.ap()[: n_full_u * 128, :].rearrange(
                    "(t p) one -> p (t one)", p=128
                ),
                su_sb[:, :n_full_u],
            )
            nc.sync.dma_start(
                s_uc.ap()[n_full_u * 128:, :],
                su_sb[:rem_u, n_full_u:n_full_u + 1],
            )
            n_full_i = I_SH // 128              # 97
            rem_i = I_SH - n_full_i * 128       # 84
            nc.sync.dma_start(
                s_ic.ap()[: n_full_i * 128, :].rearrange(
                    "(t p) one -> p (t one)", p=128
                ),
                si_sb[:, :n_full_i],
            )
            nc.sync.dma_start(
                s_ic.ap()[n_full_i * 128:, :],
                si_sb[:rem_i, n_full_i:n_full_i + 1],
            )

            nc.gpsimd.collective_compute(
                "AllGather", mybir.AluOpType.bypass,
                replica_groups=groups, ins=[s_uc.ap()], outs=[s_uf.ap()],
            )
            nc.gpsimd.collective_compute(
                "AllGather", mybir.AluOpType.bypass,
                replica_groups=groups, ins=[s_ic.ap()], outs=[s_if.ap()],
            )

            # ---- phase 2: per-element indirect gathers ----
            idxu_t = gpool.tile([128, N_GI * COLS], i32)
            nc.sync.dma_start(idxu_t[:], idxu.ap())
            idxi_t = gpool.tile([128, N_GI * COLS], i32)
            nc.sync.dma_start(idxi_t[:], idxi.ap())

            # all user gathers FIRST: gpsimd runs in order, and the user
            # table is ready well before the item table. Each batch sits in
            # a critical section with a manual completion semaphore so the
            # Q7 generates descriptors back-to-back instead of waiting for
            # each gather's DMA to drain.
            gu_t = gpool.tile([128, CP], f32)
            gi_t = gpool.tile([128, CP], f32)
            sem_g = nc.alloc_semaphore("gsem")
            with tc.tile_critical(no_gpsimd_drain=True):
                for p in range(N_GI):
                    nc.gpsimd.indirect_dma_start(
                        out=gu_t[p:p + 1, :].rearrange(
                            "one (c x) -> one c x", x=1
                        ),
                        out_offset=None,
                        in_=s_uf.ap(),
                        in_offset=IndirectOffsetOnAxis(
                            ap=idxu_t[:, p * COLS:(p + 1) * COLS], axis=0
                        ),
                    ).then_inc(sem_g, 16)
            with tc.tile_critical(no_gpsimd_drain=True):
                for p in range(N_GI):
                    nc.gpsimd.indirect_dma_start(
                        out=gi_t[p:p + 1, :].rearrange(
                            "one (c x) -> one c x", x=1
                        ),
                        out_offset=None,
                        in_=s_if.ap(),
                        in_offset=IndirectOffsetOnAxis(
                            ap=idxi_t[:, p * COLS:(p + 1) * COLS], axis=0
                        ),
                    ).then_inc(sem_g, 16)

            sc_t = gpool.tile([N_GI, CP], f32)
            with tc.tile_critical():
                nc.vector.wait_ge(sem_g, 2 * N_GI * 16)
                nc.vector.tensor_add(
                    sc_t[:], gu_t[:N_GI, :], gi_t[:N_GI, :]
                )
            nc.sync.dma_start(out.ap(), sc_t[:])

    nc.compile()
    _CACHE["nc"] = nc
    return nc


def _wrap_idx(vals):
    """Lay a per-core int32 index stream out for the gather instructions.

    Instruction p consumes its [128, COLS] idx slice in spray order
    (partition-fastest), writing element j of its row; so slice columns
    hold vals[p*CP : (p+1)*CP] reshaped (COLS, 128) transposed.
    """
    full = np.zeros(E_PAD, dtype=np.int32)
    full[: len(vals)] = vals
    outm = np.empty((128, N_GI * COLS), dtype=np.int32)
    for p in range(N_GI):
        seg = full[p * CP:(p + 1) * CP]
        outm[:, p * COLS:(p + 1) * COLS] = seg.reshape(COLS, 128).T
    return outm


def _make_in_maps(inputs):
    z_user = np.ascontiguousarray(np.asarray(inputs["z_user"], dtype=np.float32))
    z_item = np.ascontiguousarray(np.asarray(inputs["z_item"], dtype=np.float32))
    src = np.asarray(inputs["edge_src"]).astype(np.int32)
    dst = np.asarray(inputs["edge_dst"]).astype(np.int32)
    w_user = np.asarray(inputs["w_user"], dtype=np.float32)
    w_item = np.asarray(inputs["w_item"], dtype=np.float32)
    b_user = np.asarray(inputs["b_user"], dtype=np.float32).reshape(HALF, 1)
    b_item = np.asarray(inputs["b_item"], dtype=np.float32).reshape(HALF, 1)
    w_out = np.asarray(inputs["w_out"], dtype=np.float32)
    b_out = np.asarray(inputs["b_out"], dtype=np.float32).reshape(1, 1)
    wo_u = w_out[0, :HALF].reshape(HALF, 1).copy()
    wo_i = w_out[0, HALF:].reshape(HALF, 1).copy()

    in_maps = []
    for k in range(N_CORES):
        zu_k = np.zeros((U_PAD, H), dtype=np.float32)
        zu_k[:U_SH] = z_user[k * U_SH:(k + 1) * U_SH]
        zi_k = np.zeros((I_PAD, H), dtype=np.float32)
        zi_k[:I_SH] = z_item[k * I_SH:(k + 1) * I_SH]
        in_maps.append({
            "zu": zu_k,
            "zi": zi_k,
            "w_user": w_user,
            "w_item": w_item,
            "wo_u": wo_u,
            "wo_i": wo_i,
            "b_user": b_user,
            "b_item": b_item,
            "b_out": b_out,
            "idxu": _wrap_idx(src[k * E_SH:(k + 1) * E_SH]),
            "idxi": _wrap_idx(dst[k * E_SH:(k + 1) * E_SH]),
        })
    return in_maps


def _run(inputs, trace=False):
    from concourse.bass_utils import run_bass_kernel_spmd

    nc = _build()
    in_maps = _make_in_maps(inputs)
    res = run_bass_kernel_spmd(
        nc, in_maps, core_ids=list(range(N_CORES)), trace=trace
    )
    parts = [res.results[k]["out"].reshape(-1)[:E_SH] for k in range(N_CORES)]
    full = np.concatenate(parts).reshape(E, 1).astype(np.float32)
    return full, res


def kernel(**inputs):
    full, _ = _run(inputs, trace=False)
    return full


# revision 6
# speedup vs baseline: 1.1753x; 1.0457x over previous
"""Trainium2 Bass kernel for ContextAwareArtRecSys (gnn_message_passing).

Math fold: the reference is
    score[e] = concat(z_u[src] @ Wu.T + bu, z_i[dst] @ Wi.T + bi) @ wo.T + bo
Everything after the gather is linear, so with
    vu = wo[:, :128] @ Wu          (256-vector)
    vi = wo[:, 128:] @ Wi          (256-vector)
    c  = wo[:, :128]@bu + wo[:, 128:]@bi + bo   (scalar)
we have score[e] = (z_u @ vu)[src] + (z_i @ vi)[dst] + c.

Device plan per core k of 8 (SPMD):
  1. fold vu/vi/c on PE from the raw weights (replicated inputs).
  2. matvec s_u = z_u_shard @ vu + c, s_i = z_i_shard @ vi on DVE while
     the z shards stream in (z is sharded 8-ways by node).
  3. AllGather the scalar tables to DRAM (50000 + 100000 f32).
  4. per-element indirect-DMA gather of s_u[src_e] and s_i[dst_e] for the
     core's 1/8 slice of the edge list, add, store.
All heavy work is on-device; the host only slices/casts/pads/lays out
shards and concatenates the output.
"""

import numpy as np

N_CORES = 8
N_USERS, N_ITEMS, E, H = 50000, 100000, 500000, 256
HALF = H // 2

U_SH = N_USERS // N_CORES          # 6250 users per core
I_SH = N_ITEMS // N_CORES          # 12500 items per core
E_SH = E // N_CORES                # 62500 edges per core
U_PAD = 6272                       # 49 * 128
I_PAD = 12544                      # 98 * 128
U_TILES = U_PAD // 128             # 49
I_TILES = I_PAD // 128             # 98

N_GI = 16                          # gather instructions per table
CP = 3968                          # elements gathered per instruction (31*128)
COLS = CP // 128                   # 31 idx columns per instruction
E_PAD = N_GI * CP                  # 63488 padded edge slots per core

_CACHE = {}


def _build():
    if "nc" in _CACHE:
        return _CACHE["nc"]
    import concourse.bass as bass
    import concourse.tile as tile
    import concourse.mybir as mybir
    from concourse import bacc
    from concourse.bass import IndirectOffsetOnAxis

    f32 = mybir.dt.float32
    i32 = mybir.dt.int32

    nc = bacc.Bacc("TRN2", target_bir_lowering=False, debug=False,
                   num_devices=N_CORES)

    zu = nc.dram_tensor("zu", [U_PAD, H], f32, kind="ExternalInput")
    zi = nc.dram_tensor("zi", [I_PAD, H], f32, kind="ExternalInput")
    w_user = nc.dram_tensor("w_user", [HALF, H], f32, kind="ExternalInput")
    w_item = nc.dram_tensor("w_item", [HALF, H], f32, kind="ExternalInput")
    wo_u = nc.dram_tensor("wo_u", [HALF, 1], f32, kind="ExternalInput")
    wo_i = nc.dram_tensor("wo_i", [HALF, 1], f32, kind="ExternalInput")
    b_user = nc.dram_tensor("b_user", [HALF, 1], f32, kind="ExternalInput")
    b_item = nc.dram_tensor("b_item", [HALF, 1], f32, kind="ExternalInput")
    b_out = nc.dram_tensor("b_out", [1, 1], f32, kind="ExternalInput")
    idxu = nc.dram_tensor("idxu", [128, N_GI * COLS], i32, kind="ExternalInput")
    idxi = nc.dram_tensor("idxi", [128, N_GI * COLS], i32, kind="ExternalInput")
    out = nc.dram_tensor("out", [N_GI, CP], f32, kind="ExternalOutput")

    s_uc = nc.dram_tensor("s_uc", [U_SH, 1], f32)
    s_ic = nc.dram_tensor("s_ic", [I_SH, 1], f32)
    s_uf = nc.dram_tensor("s_uf", [N_USERS, 1], f32, addr_space="Shared")
    s_if = nc.dram_tensor("s_if", [N_ITEMS, 1], f32, addr_space="Shared")

    groups = [list(range(N_CORES))]

    with tile.TileContext(nc) as tc:
        with (
            tc.tile_pool(name="consts", bufs=1) as consts,
            tc.tile_pool(name="zpool", bufs=4) as zpool,
            tc.tile_pool(name="spool", bufs=1) as spool,
            tc.tile_pool(name="gpool", bufs=1) as gpool,
            tc.tile_pool(name="psum", bufs=2, space="PSUM") as psum,
        ):
            # ---- fold vu / vi / c on PE ----
            wu_t = consts.tile([HALF, H], f32)
            nc.sync.dma_start(wu_t[:], w_user.ap())
            wi_t = consts.tile([HALF, H], f32)
            nc.sync.dma_start(wi_t[:], w_item.ap())
            wou_t = consts.tile([HALF, 1], f32)
            nc.sync.dma_start(wou_t[:], wo_u.ap())
            woi_t = consts.tile([HALF, 1], f32)
            nc.sync.dma_start(woi_t[:], wo_i.ap())
            bu_t = consts.tile([HALF, 1], f32)
            nc.sync.dma_start(bu_t[:], b_user.ap())
            bi_t = consts.tile([HALF, 1], f32)
            nc.sync.dma_start(bi_t[:], b_item.ap())
            bo_t = consts.tile([1, 1], f32)
            nc.sync.dma_start(bo_t[:], b_out.ap())

            # replicate wo halves across the free dim: rep[k, m] = wo[k]
            ones_kk = consts.tile([HALF, HALF], f32)
            nc.vector.memset(ones_kk[:], 1.0)
            wou_rep = consts.tile([HALF, HALF], f32)
            nc.vector.tensor_scalar_mul(wou_rep[:], ones_kk[:], wou_t[:])
            woi_rep = consts.tile([HALF, HALF], f32)
            nc.vector.tensor_scalar_mul(woi_rep[:], ones_kk[:], woi_t[:])

            # vu/vi broadcast across all 128 partitions: [128, H] PSUM
            vu_ps = psum.tile([HALF, H], f32, tag="vps")
            nc.tensor.matmul(vu_ps[:], wou_rep[:], wu_t[:], start=True, stop=True)
            vu_t = consts.tile([HALF, H], f32)
            nc.vector.tensor_copy(vu_t[:], vu_ps[:])
            vi_ps = psum.tile([HALF, H], f32, tag="vps")
            nc.tensor.matmul(vi_ps[:], woi_rep[:], wi_t[:], start=True, stop=True)
            vi_t = consts.tile([HALF, H], f32)
            nc.vector.tensor_copy(vi_t[:], vi_ps[:])

            # c = wo_u . b_user + wo_i . b_item + b_out, broadcast to [128,1]
            ones_k1 = consts.tile([HALF, 128], f32)
            nc.vector.memset(ones_k1[:], 1.0)
            cu_ps = psum.tile([128, 1], f32, tag="cps")
            bub = consts.tile([HALF, 128], f32)
            nc.vector.tensor_scalar_mul(bub[:], ones_k1[:], bu_t[:])
            bib = consts.tile([HALF, 128], f32)
            nc.vector.tensor_scalar_mul(bib[:], ones_k1[:], bi_t[:])
            # cu_ps[m, 0] = sum_k bub[k, m] * wo_u[k]  (same for all m)
            nc.tensor.matmul(cu_ps[:], bub[:], wou_t[:], start=True, stop=False)
            nc.tensor.matmul(cu_ps[:], bib[:], woi_t[:], start=False, stop=False)
            # += 1 * b_out via a K=1 matmul (ones row as lhsT)
            nc.tensor.matmul(
                cu_ps[:], ones_k1[0:1, :], bo_t[:], start=False, stop=True
            )
            c_t = consts.tile([128, 1], f32)
            nc.vector.tensor_copy(c_t[:], cu_ps[:])

            # ---- phase 1: s tables (DVE matvec over streamed z tiles) ----
            su_sb = spool.tile([128, U_TILES], f32)
            for t in range(U_TILES):
                zt = zpool.tile([128, H], f32, tag="z")
                nc.sync.dma_start(zt[:], zu.ap()[t * 128:(t + 1) * 128, :])
                pr = zpool.tile([128, H], f32, tag="prod")
                nc.vector.tensor_mul(pr[:], zt[:], vu_t[:])
                nc.vector.reduce_sum(
                    su_sb[:, t:t + 1], pr[:], axis=mybir.AxisListType.X
                )
            # add folded constant c into the user table
            nc.vector.tensor_scalar_add(su_sb[:], su_sb[:], c_t[:])

            # store + allgather the user table immediately so the user
            # gathers can start while the item table is still being built
            # s_uc[n] for n = 128*t + p  <=>  sbuf [p, t]
            n_full_u = U_SH // 128              # 48 full tiles
            rem_u = U_SH - n_full_u * 128       # 106
            nc.sync.dma_start(
                s_uc.ap()[: n_full_u * 128, :].rearrange(
                    "(t p) one -> p (t one)", p=128
                ),
                su_sb[:, :n_full_u],
            )
            nc.sync.dma_start(
                s_uc.ap()[n_full_u * 128:, :],
                su_sb[:rem_u, n_full_u:n_full_u + 1],
            )
            nc.gpsimd.collective_compute(
                "AllGather", mybir.AluOpType.bypass,
                replica_groups=groups, ins=[s_uc.ap()], outs=[s_uf.ap()],
            )

            si_sb = spool.tile([128, I_TILES], f32)
            for t in range(I_TILES):
                zt = zpool.tile([128, H], f32, tag="z")
                nc.sync.dma_start(zt[:], zi.ap()[t * 128:(t + 1) * 128, :])
                pr = zpool.tile([128, H], f32, tag="prod")
                nc.vector.tensor_mul(pr[:], zt[:], vi_t[:])
                nc.vector.reduce_sum(
                    si_sb[:, t:t + 1], pr[:], axis=mybir.AxisListType.X
                )

            # ---- store item chunk to DRAM (node-major order) ----
            n_full_i = I_SH // 128              # 97
            rem_i = I_SH - n_full_i * 128       # 84
            nc.sync.dma_start(
                s_ic.ap()[: n_full_i * 128, :].rearrange(
                    "(t p) one -> p (t one)", p=128
                ),
                si_sb[:, :n_full_i],
            )
            nc.sync.dma_start(
                s_ic.ap()[n_full_i * 128:, :],
                si_sb[:rem_i, n_full_i:n_full_i + 1],
            )

            nc.gpsimd.collective_compute(
                "AllGather", mybir.AluOpType.bypass,
                replica_groups=groups, ins=[s_ic.ap()], outs=[s_if.ap()],
            )

            # ---- phase 2: per-element indirect gathers ----
            idxu_t = gpool.tile([128, N_GI * COLS], i32)
            nc.sync.dma_start(idxu_t[:], idxu.ap())
            idxi_t = gpool.tile([128, N_GI * COLS], i32)
            nc.sync.dma_start(idxi_t[:], idxi.ap())

            # all user gathers FIRST: gpsimd runs in order, and the user
            # table is ready well before the item table. Each batch sits in
            # a critical section with a manual completion semaphore so the
            # Q7 generates descriptors back-to-back instead of waiting for
            # each gather's DMA to drain.
            gu_t = gpool.tile([128, CP], f32)
            gi_t = gpool.tile([128, CP], f32)
            sem_g = nc.alloc_semaphore("gsem")
            with tc.tile_critical(no_gpsimd_drain=True):
                for p in range(N_GI):
                    nc.gpsimd.indirect_dma_start(
                        out=gu_t[p:p + 1, :].rearrange(
                            "one (c x) -> one c x", x=1
                        ),
                        out_offset=None,
                        in_=s_uf.ap(),
                        in_offset=IndirectOffsetOnAxis(
                            ap=idxu_t[:, p * COLS:(p + 1) * COLS], axis=0
                        ),
                    ).then_inc(sem_g, 16)
            with tc.tile_critical(no_gpsimd_drain=True):
                for p in range(N_GI):
                    nc.gpsimd.indirect_dma_start(
                        out=gi_t[p:p + 1, :].rearrange(
                            "one (c x) -> one c x", x=1
                        ),
                        out_offset=None,
                        in_=s_if.ap(),
                        in_offset=IndirectOffsetOnAxis(
                            ap=idxi_t[:, p * COLS:(p + 1) * COLS], axis=0
                        ),
                    ).then_inc(sem_g, 16)

            sc_t = gpool.tile([N_GI, CP], f32)
            with tc.tile_critical():
                nc.vector.wait_ge(sem_g, 2 * N_GI * 16)
                nc.vector.tensor_add(
                    sc_t[:], gu_t[:N_GI, :], gi_t[:N_GI, :]
                )
            nc.sync.dma_start(out.ap(), sc_t[:])

    nc.compile()
    _CACHE["nc"] = nc
    return nc


def _wrap_idx(vals):
    """Lay a per-core int32 index stream out for the gather instructions.

    Instruction p consumes its [128, COLS] idx slice in spray order
    (partition-fastest), writing element j of its row; so slice columns
    hold vals[p*CP : (p+1)*CP] reshaped (COLS, 128) transposed.
    """
    full = np.zeros(E_PAD, dtype=np.int32)
    full[: len(vals)] = vals
    outm = np.empty((128, N_GI * COLS), dtype=np.int32)
    for p in range(N_GI):
        seg = full[p * CP:(p + 1) * CP]
        outm[:, p * COLS:(p + 1) * COLS] = seg.reshape(COLS, 128).T
    return outm


def _make_in_maps(inputs):
    z_user = np.ascontiguousarray(np.asarray(inputs["z_user"], dtype=np.float32))
    z_item = np.ascontiguousarray(np.asarray(inputs["z_item"], dtype=np.float32))
    src = np.asarray(inputs["edge_src"]).astype(np.int32)
    dst = np.asarray(inputs["edge_dst"]).astype(np.int32)
    w_user = np.asarray(inputs["w_user"], dtype=np.float32)
    w_item = np.asarray(inputs["w_item"], dtype=np.float32)
    b_user = np.asarray(inputs["b_user"], dtype=np.float32).reshape(HALF, 1)
    b_item = np.asarray(inputs["b_item"], dtype=np.float32).reshape(HALF, 1)
    w_out = np.asarray(inputs["w_out"], dtype=np.float32)
    b_out = np.asarray(inputs["b_out"], dtype=np.float32).reshape(1, 1)
    wo_u = w_out[0, :HALF].reshape(HALF, 1).copy()
    wo_i = w_out[0, HALF:].reshape(HALF, 1).copy()

    in_maps = []
    for k in range(N_CORES):
        zu_k = np.zeros((U_PAD, H), dtype=np.float32)
        zu_k[:U_SH] = z_user[k * U_SH:(k + 1) * U_SH]
        zi_k = np.zeros((I_PAD, H), dtype=np.float32)
        zi_k[:I_SH] = z_item[k * I_SH:(k + 1) * I_SH]
        in_maps.append({
            "zu": zu_k,
            "zi": zi_k,
            "w_user": w_user,
            "w_item": w_item,
            "wo_u": wo_u,
            "wo_i": wo_i,
            "b_user": b_user,
            "b_item": b_item,
            "b_out": b_out,
            "idxu": _wrap_idx(src[k * E_SH:(k + 1) * E_SH]),
            "idxi": _wrap_idx(dst[k * E_SH:(k + 1) * E_SH]),
        })
    return in_maps


def _run(inputs, trace=False):
    from concourse.bass_utils import run_bass_kernel_spmd

    nc = _build()
    in_maps = _make_in_maps(inputs)
    res = run_bass_kernel_spmd(
        nc, in_maps, core_ids=list(range(N_CORES)), trace=trace
    )
    parts = [res.results[k]["out"].reshape(-1)[:E_SH] for k in range(N_CORES)]
    full = np.concatenate(parts).reshape(E, 1).astype(np.float32)
    return full, res


def kernel(**inputs):
    full, _ = _run(inputs, trace=False)
    return full
